# revision 13
# baseline (speedup 1.0000x reference)
"""DiffAttention kernel for 8 TRN2 NeuronCores (Bass/Tile).

Reference computation (see problem): x [1,128,32,32,32] is stride-2
subsampled to xs [128, N=4096 tokens]; qkv = w_qkv @ xs per head
(4 heads, head_dim 32, split into two halves of 16 for the two
softmaxes); diff_attn = softmax(q1k1) - 0.1*softmax(q2k2); out = diff
attn @ v, reshaped back to [1,128,16,16,16].

Sharding: tensor-parallel over (head, query-half) = 8 shards, one per
core. Each core computes its head's full K/V over all 4096 tokens and
attention for its 2048 queries.

Per-core dataflow (all on-chip, flash-style, no NxN HBM traffic):
  - scores are computed TRANSPOSED, sT[m,n] = k^T q, so the softmax
    denominator can be folded into the AV matmul via a ones-column
    appended to v^T; k1/q1 live on partition strip 32:48 and k2/q2 on
    64:80 so the two score matmuls row-pair on the array.
  - exp is SPLIT across two engines so neither paces the loop: ACT
    exps the s1 half (exact, feeds the dominant softmax), DVE exps the
    s2 half with the Schraudolph bit trick -- one tensor_scalar
    computing int16(round(128*log2e*scale*s + 16248.59)) whose bits
    ARE bf16(exp(s*scale)) to within +-3%; that error enters the
    output attenuated by lambda=0.1.
  - the m-loop is processed in GROUPS of 8: [8 score pairs + exps]
    then [the previous group's 8 av pairs].  Long uniform runs let the
    PE's fast streaming mode engage (hw-measured: alternating
    score/av pairs every iteration streams at ~427 ns/pair, batched
    runs reach ~216 ns/pair); all projection matmuls and the finalize
    transposes are confined to group boundaries for the same reason.
  - AV: out^T[d,n] accumulated over m-chunks in PSUM; AV1 at psum
    partitions 0:33, AV2 at 64:97 (col-tiled pair).
  - finalize: PE-transpose av -> [n,33], per-partition reciprocal of
    the sum column, combine out = av1/s1 - 0.1*av2/s2 on DVE.
"""

import math

import numpy as np
import ml_dtypes

import concourse.bass as bass
import concourse.mybir as mybir
import concourse.tile as tile
from concourse import bacc
from concourse.bass import ts, ds
from concourse.bass_utils import run_bass_kernel_spmd

BF16 = mybir.dt.bfloat16
F32 = mybir.dt.float32
I16 = mybir.dt.int16
NP_BF16 = ml_dtypes.bfloat16

C = 128          # channels
HEADS = 4
HD = 32          # head_dim
DH = 16          # d_half
LAMBDA = 0.1
SCALE = HD ** -0.5
R = 2
N_CORES = 8
N = 4096         # tokens after subsample
NQ = N // 2      # queries per core

# Schraudolph constants: int16 bits of bf16(2^y) ~= 128*y + 128*(127-c),
# c = 0.0579297 balances the max relative error at ~+-2.98%; the DVE
# f32->int16 store rounds to nearest (hw-verified).
SCH_A = 128.0 * math.log2(math.e) * SCALE
SCH_B = 128.0 * (127.0 - 0.0579297)

# per-iteration exp column split (of the 1024-col sj tile): ACT does
# [0:CA], DVE does [CA:1024].  511 so av1's rhs [0:512] overlaps the
# DVE range and carries both exp semaphores.  During j-block 0 the DVE
# also absorbs the k projection copies, so ACT takes more columns.
CA = 511
CA_J0 = 608

GB = 8           # m-chunks per group
MC = N // 128    # 32 m-chunks
NG = MC // GB    # 4 groups per j-block
NJ = NQ // 512   # 4 j-blocks per core
NBS = 1024       # queries per av accumulator block (2 j-blocks)

# weight tensor column layout (w input, [128, 96]):
WV = slice(0, 32)     # w_v^T   (rhs of vT matmuls)
WK1 = slice(32, 48)   # w_k1^T
WK2 = slice(48, 64)   # w_k2^T
WQ1 = slice(64, 80)   # w_q1^T
WQ2 = slice(80, 96)   # w_q2^T


def build_nc(NT=N, NQL=NQ):
    """Build the SPMD Bass program for one core = (head, query-half).

    Per-core inputs:
      xs    [128, NT]   bf16  all tokens, channel-major (for K and V)
      xq    [128, NQL]  bf16  this core's query tokens
      w     [128, 96]   bf16  columns per WV/WK1/WK2/WQ1/WQ2 slices
      ident [128, 33]   f32   identity blocks at partitions 0:33, 64:97
    Output:
      out   [NQL, 32]   f32   attention output (n, d) for the queries
    """
    Exp = mybir.ActivationFunctionType.Exp

    nc = bacc.Bacc()
    xs_d = nc.declare_dram_parameter("xs", [C, NT], BF16, isOutput=False)
    xq_d = nc.declare_dram_parameter("xq", [C, NQL], BF16, isOutput=False)
    w_d = nc.declare_dram_parameter("w", [C, 96], BF16, isOutput=False)
    id_d = nc.declare_dram_parameter("ident", [C, 33], F32, isOutput=False)
    out_d = nc.declare_dram_parameter("out", [NQL, HD], F32, isOutput=True)

    with tile.TileContext(nc) as tc:
        with (
            tc.tile_pool(name="consts", bufs=1) as consts,
            tc.tile_pool(name="mains", bufs=1) as mains,
        ):
            w_sb = consts.tile([C, 96], BF16)
            nc.sync.dma_start(out=w_sb[:, :], in_=w_d[:, :])
            id_sb = consts.tile([C, 33], F32)
            nc.sync.dma_start(out=id_sb[:, :], in_=id_d[:, :])

            def chunked_dma(eng, dst, src, total):
                sizes, rem = [], total
                for sz in (512, 512, 1024):
                    if rem >= sz:
                        sizes.append(sz)
                        rem -= sz
                while rem > 0:
                    sz = 2048 if rem >= 2048 else 512
                    sizes.append(sz)
                    rem -= sz
                off = 0
                for sz in sizes:
                    eng.dma_start(out=dst[:, ds(off, sz)],
                                  in_=src[:, ds(off, sz)])
                    off += sz

            xs_sb = mains.tile([C, NT], BF16)
            chunked_dma(nc.gpsimd, xs_sb, xs_d, NT)
            xq_sb = mains.tile([C, NQL], BF16)
            chunked_dma(nc.scalar, xq_sb, xq_d, NQL)

            kk_sb = mains.tile([C, NT], BF16)    # parts 32:48 k1, 64:80 k2
            qq_sb = mains.tile([C, NQL], BF16)   # parts 32:48 q1, 64:80 q2
            vTa_sb = mains.tile([C, MC * 33], BF16)  # per chunk: v^T | ones
            av_sb = mains.tile([C, 2 * NBS], F32)  # parts 0:33 AV1|s1, 64:97 AV2|s2
            out_sb = mains.tile([C, (NQL // 128) * HD], F32)

            nc.vector.memset(vTa_sb[:, :], 1.0)

            with (
                tc.tile_pool(name="sj_ps", bufs=3, space="PSUM") as spool,
                tc.tile_pool(name="av_ps", bufs=1, space="PSUM") as avpool,
                tc.tile_pool(name="e_sb", bufs=18) as epool,
                tc.tile_pool(name="fin_sb", bufs=2) as fsb,
            ):
                def project_q(t):
                    # q chunk t = queries for j-block t
                    ps_q = spool.tile([C, 1024], F32, tag="sj", name="ps_q")
                    nc.tensor.matmul(ps_q[32:48, 0:512], lhsT=w_sb[:, WQ1],
                                     rhs=xq_sb[:, ts(t, 512)],
                                     start=True, stop=True)
                    nc.tensor.matmul(ps_q[64:80, 0:512], lhsT=w_sb[:, WQ2],
                                     rhs=xq_sb[:, ts(t, 512)],
                                     start=True, stop=True)
                    nc.vector.tensor_copy(qq_sb[32:48, ts(t, 512)],
                                          ps_q[32:48, 0:512])
                    nc.vector.tensor_copy(qq_sb[64:80, ts(t, 512)],
                                          ps_q[64:80, 0:512])

                def project_kv(t):
                    # k chunk t = keys for m-chunks 4t..4t+3
                    ps_kv = spool.tile([C, 1024], F32, tag="sj", name="ps_kv")
                    nc.tensor.matmul(ps_kv[32:48, 0:512], lhsT=w_sb[:, WK1],
                                     rhs=xs_sb[:, ts(t, 512)],
                                     start=True, stop=True)
                    nc.tensor.matmul(ps_kv[64:80, 0:512], lhsT=w_sb[:, WK2],
                                     rhs=xs_sb[:, ts(t, 512)],
                                     start=True, stop=True)
                    nc.vector.tensor_copy(kk_sb[32:48, ts(t, 512)],
                                          ps_kv[32:48, 0:512])
                    nc.vector.tensor_copy(kk_sb[64:80, ts(t, 512)],
                                          ps_kv[64:80, 0:512])

                def project_vt(m):
                    ps_vt = spool.tile([C, 1024], F32, tag="sj", name="ps_vt")
                    nc.tensor.matmul(ps_vt[:, 0:HD], lhsT=xs_sb[:, ts(m, 128)],
                                     rhs=w_sb[:, WV], start=True, stop=True)
                    nc.scalar.copy(vTa_sb[:, ds(m * 33, HD)], ps_vt[:, 0:HD])

                def finalize_nb(nb):
                    # transpose av -> [n, 33], reciprocal of the sum
                    # column, combine out = av1/s1 - 0.1*av2/s2 on DVE
                    CQ = NBS // 128  # 8 query chunks of 128
                    psT1 = spool.tile([C, 1024], F32, tag="sj", name="psT1")
                    psT2 = spool.tile([C, 1024], F32, tag="sj", name="psT2")
                    for cq in range(CQ):
                        gq = nb * CQ + cq
                        nc.tensor.transpose(psT1[:, ds(cq * 64, 33)],
                                            av_sb[0:33, ts(gq, 128)],
                                            id_sb[0:33, :])
                        nc.tensor.transpose(psT2[:, ds(cq * 64, 33)],
                                            av_sb[64:97, ts(gq, 128)],
                                            id_sb[64:97, :])
                    r1_sb = fsb.tile([C, CQ], F32, tag="r1")
                    r2_sb = fsb.tile([C, CQ], F32, tag="r2")
                    sum1 = psT1[:, 0:CQ * 64].rearrange(
                        "p (c x) -> p c x", x=64)[:, :, 32:33]
                    sum2 = psT2[:, 0:CQ * 64].rearrange(
                        "p (c x) -> p c x", x=64)[:, :, 32:33]
                    nc.vector.reciprocal(r1_sb[:, :, None], sum1)
                    nc.vector.reciprocal(r2_sb[:, :, None], sum2)
                    nc.vector.tensor_scalar_mul(r2_sb[:, :], r2_sb[:, :],
                                                -LAMBDA)
                    o1_sb = fsb.tile([C, CQ * HD], F32, tag="o1")
                    o2_sb = fsb.tile([C, CQ * HD], F32, tag="o2")
                    av1t = psT1[:, 0:CQ * 64].rearrange(
                        "p (c x) -> p c x", x=64)[:, :, 0:32]
                    av2t = psT2[:, 0:CQ * 64].rearrange(
                        "p (c x) -> p c x", x=64)[:, :, 0:32]
                    o1_v = o1_sb[:, :].rearrange("p (c d) -> p c d", d=HD)
                    o2_v = o2_sb[:, :].rearrange("p (c d) -> p c d", d=HD)
                    nc.vector.tensor_tensor(
                        o1_v, av1t,
                        r1_sb[:, :, None].to_broadcast((C, CQ, HD)),
                        mybir.AluOpType.mult)
                    nc.vector.tensor_tensor(
                        o2_v, av2t,
                        r2_sb[:, :, None].to_broadcast((C, CQ, HD)),
                        mybir.AluOpType.mult)
                    nc.vector.tensor_tensor(
                        out_sb[:, ds(nb * CQ * HD, CQ * HD)],
                        o1_sb[:, :], o2_sb[:, :], mybir.AluOpType.add)
                    out_view = out_d[:, :].rearrange("(c p) d -> p c d", p=C)
                    nc.sync.dma_start(
                        out=out_view[:, nb * CQ:(nb + 1) * CQ, :],
                        in_=out_sb[:, ds(nb * CQ * HD, CQ * HD)]
                            .rearrange("p (c d) -> p c d", d=HD),
                    )

                # minimal chain to the first scores: k chunks 0,1 and
                # the first j-block's queries
                project_kv(0)
                project_q(0)
                project_kv(1)

                pending_av = None      # av batch closure of previous group
                pending_fin = None     # finalize closure of previous n-block

                for j in range(NJ):
                    nb = j // 2
                    for g in range(NG):
                        # ---- previous group's av batch FIRST: its
                        # earlier heap priority makes it one uniform
                        # burst at the group start, and leaves nothing
                        # for the scheduler to interleave into the
                        # s batch's exp-paced stalls
                        if pending_av is not None:
                            pending_av()
                            pending_av = None
                        # ---- boundary work (all PE disturbances live
                        # here): projections one group ahead + finalize
                        if j == 0:
                            if g >= 1:
                                project_kv(2 * g)
                                project_kv(2 * g + 1)
                            # v^T for this group's m-range (consumed by
                            # this group's av batch)
                            for m in range(g * GB, g * GB + GB):
                                project_vt(m)
                        if j == 0 and g == 2:
                            project_q(1)
                        if j == 1 and g == 2:
                            project_q(2)
                        if j == 2 and g == 0 and pending_fin is not None:
                            pending_fin()
                            pending_fin = None
                        if j == 2 and g == 2:
                            project_q(3)

                        # ---- s batch: 8 score pairs + split exps
                        ca = CA_J0 if j == 0 else CA
                        e_tiles = []
                        for m in range(g * GB, g * GB + GB):
                            nsl = ds(j * 512, 512)
                            sj_ps = spool.tile([C, 1024], F32, tag="sj")
                            nc.tensor.matmul(sj_ps[:, 0:512],
                                             lhsT=kk_sb[32:48, ts(m, 128)],
                                             rhs=qq_sb[32:48, nsl],
                                             start=True, stop=True)
                            nc.tensor.matmul(sj_ps[:, 512:1024],
                                             lhsT=kk_sb[64:80, ts(m, 128)],
                                             rhs=qq_sb[64:80, nsl],
                                             start=True, stop=True)
                            e_sb = epool.tile([C, 1024], BF16, tag="e")
                            nc.scalar.activation(e_sb[:, 0:ca],
                                                 sj_ps[:, 0:ca], Exp,
                                                 scale=SCALE)
                            nc.vector.tensor_scalar(
                                e_sb[:, ca:1024].bitcast(I16),
                                sj_ps[:, ca:1024], SCH_A, SCH_B,
                                mybir.AluOpType.mult, mybir.AluOpType.add)
                            e_tiles.append((m, e_sb))

                        def av_batch(e_tiles=e_tiles, j=j, av_ps=None):
                            for m, e_sb in e_tiles:
                                first, last = (m % MC == 0), (m % MC == MC - 1)
                                nc.tensor.matmul(
                                    av_ps[0:33, ts(j % 2, 512)],
                                    lhsT=vTa_sb[:, ds(m * 33, 33)],
                                    rhs=e_sb[:, 0:512],
                                    start=first, stop=last,
                                    skip_group_check=True)
                                nc.tensor.matmul(
                                    av_ps[64:97, ts(j % 2, 512)],
                                    lhsT=vTa_sb[:, ds(m * 33, 33)],
                                    rhs=e_sb[:, 512:1024],
                                    start=first, stop=last,
                                    skip_group_check=True)
                        if j % 2 == 0 and g == 0:
                            av_tile = avpool.tile([C, NBS], F32, tag="av")
                        pending_av = (lambda f=av_batch, t=av_tile:
                                      f(av_ps=t))

                    # ---- end of j-block
                    if j % 2 == 1:
                        # flush the block's last av batch, then drain
                        pending_av()
                        pending_av = None
                        nc.vector.tensor_copy(
                            av_sb[0:33, ds(nb * NBS, NBS)], av_tile[0:33, :])
                        nc.scalar.copy(
                            av_sb[64:97, ds(nb * NBS, NBS)], av_tile[64:97, :])
                        if j == NJ - 1:
                            finalize_nb(nb)
                        else:
                            pending_fin = (lambda nb=nb: finalize_nb(nb))

    nc.compile()
    return nc


def make_identity_input():
    ident = np.zeros((C, 33), np.float32)
    ident[0:33, :] = np.eye(33, dtype=np.float32)
    ident[64:97, :] = np.eye(33, dtype=np.float32)
    return ident


def make_in_maps(x, w_qkv):
    """Host-side sharding: subsample, pack per-core inputs."""
    xs = np.ascontiguousarray(x[0][:, ::R, ::R, ::R]).reshape(C, N)
    xs_b = xs.astype(NP_BF16)
    ident = make_identity_input()
    in_maps = []
    for core in range(N_CORES):
        h, half = divmod(core, 2)
        wq = w_qkv[h * 96: h * 96 + 32]       # [32, 128]
        wk = w_qkv[h * 96 + 32: h * 96 + 64]
        wv = w_qkv[h * 96 + 64: h * 96 + 96]
        w = np.empty((C, 96), np.float32)
        w[:, WV] = wv.T
        w[:, WK1] = wk[0:DH].T
        w[:, WK2] = wk[DH:HD].T
        w[:, WQ1] = wq[0:DH].T
        w[:, WQ2] = wq[DH:HD].T
        in_maps.append({
            "xs": xs_b,
            "xq": np.ascontiguousarray(xs_b[:, half * NQ:(half + 1) * NQ]),
            "w": w.astype(NP_BF16),
            "ident": ident,
        })
    return in_maps


_NC_CACHE = {}


def get_nc():
    if "nc" not in _NC_CACHE:
        _NC_CACHE["nc"] = build_nc()
    return _NC_CACHE["nc"]


LAST_RESULTS = None  # BassKernelResults of the most recent kernel() call


def kernel(x, w_qkv, trace=False, **trace_kwargs):
    global LAST_RESULTS
    x = np.asarray(x)
    w_qkv = np.asarray(w_qkv)
    in_maps = make_in_maps(x, w_qkv)
    nc = get_nc()
    res = run_bass_kernel_spmd(nc, in_maps, list(range(N_CORES)),
                               trace=trace, **trace_kwargs)
    LAST_RESULTS = res
    out_hnd = np.empty((HEADS, N, HD), np.float32)
    for core in range(N_CORES):
        h, half = divmod(core, 2)
        out_hnd[h, half * NQ:(half + 1) * NQ, :] = res.results[core]["out"]
    return out_hnd.reshape(1, C, 16, 16, 16)


# revision 17
# speedup vs baseline: 1.1315x; 1.1315x over previous
"""DiffAttention kernel for 8 TRN2 NeuronCores (Bass/Tile).

Reference computation (see problem): x [1,128,32,32,32] is stride-2
subsampled to xs [128, N=4096 tokens]; qkv = w_qkv @ xs per head
(4 heads, head_dim 32, split into two halves of 16 for the two
softmaxes); diff_attn = softmax(q1k1) - 0.1*softmax(q2k2); out = diff
attn @ v, reshaped back to [1,128,16,16,16].

Sharding: tensor-parallel over (head, query-half) = 8 shards, one per
core. Each core computes its head's full K/V over all 4096 tokens and
attention for its 2048 queries.

Per-core dataflow (all on-chip, flash-style, no NxN HBM traffic):
  - scores are computed TRANSPOSED, sT[m,n] = k^T q, so the softmax
    denominator can be folded into the AV matmul via a ones-column
    appended to v^T; k1/q1 live on partition strip 32:48 and k2/q2 on
    64:80 so the two score matmuls row-pair on the array.
  - exp is SPLIT across two engines so neither paces the loop: ACT
    exps the s1 half (exact, feeds the dominant softmax), DVE exps the
    s2 half with the Schraudolph bit trick -- one tensor_scalar
    computing int16(round(128*log2e*scale*s + 16248.59)) whose bits
    ARE bf16(exp(s*scale)) to within +-3%; that error enters the
    output attenuated by lambda=0.1.
  - the m-loop is processed in GROUPS of 8: [8 score pairs + exps]
    then [the previous group's 8 av pairs].  Long uniform runs let the
    PE's fast streaming mode engage (hw-measured: alternating
    score/av pairs every iteration streams at ~427 ns/pair, batched
    runs reach ~216 ns/pair); all projection matmuls and the finalize
    transposes are confined to group boundaries for the same reason.
  - AV: out^T[d,n] accumulated over m-chunks in PSUM; AV1 at psum
    partitions 0:33, AV2 at 64:97 (col-tiled pair).
  - finalize: PE-transpose av -> [n,33], per-partition reciprocal of
    the sum column, combine out = av1/s1 - 0.1*av2/s2 on DVE.
"""

import math

import numpy as np
import ml_dtypes

import concourse.bass as bass
import concourse.mybir as mybir
import concourse.tile as tile
from concourse import bacc
from concourse.bass import ts, ds
from concourse.bass_utils import run_bass_kernel_spmd

BF16 = mybir.dt.bfloat16
F32 = mybir.dt.float32
I16 = mybir.dt.int16
NP_BF16 = ml_dtypes.bfloat16

C = 128          # channels
HEADS = 4
HD = 32          # head_dim
DH = 16          # d_half
LAMBDA = 0.1
SCALE = HD ** -0.5
R = 2
N_CORES = 8
N = 4096         # tokens after subsample
NQ = N // 2      # queries per core

# Schraudolph constants: int16 bits of bf16(2^y) ~= 128*y + 128*(127-c),
# c = 0.0579297 balances the max relative error at ~+-2.98%; the DVE
# f32->int16 store rounds to nearest (hw-verified).
SCH_A = 128.0 * math.log2(math.e) * SCALE
SCH_B = 128.0 * (127.0 - 0.0579297)

# per-iteration exp column split (of the 1024-col sj tile): ACT does
# [0:CA], DVE does [CA:1024].  511 so av1's rhs [0:512] overlaps the
# DVE range and carries both exp semaphores.  During j-block 0 the DVE
# also absorbs the k projection copies, so ACT takes more columns.
CA = 511
CA_J0 = 608

GB = 8           # m-chunks per group
MC = N // 128    # 32 m-chunks
NG = MC // GB    # 4 groups per j-block
NJ = NQ // 512   # 4 j-blocks per core
NBS = 1024       # queries per av accumulator block (2 j-blocks)

# weight tensor column layout (w input, [128, 96]):
WV = slice(0, 32)     # w_v^T   (rhs of vT matmuls)
WK1 = slice(32, 48)   # w_k1^T
WK2 = slice(48, 64)   # w_k2^T
WQ1 = slice(64, 80)   # w_q1^T
WQ2 = slice(80, 96)   # w_q2^T


def build_nc(NT=N, NQL=NQ):
    """Build the SPMD Bass program for one core = (head, query-half).

    Per-core inputs:
      xs    [128, NT]   bf16  all tokens, channel-major (for K and V)
      xq    [128, NQL]  bf16  this core's query tokens
      w     [128, 96]   bf16  columns per WV/WK1/WK2/WQ1/WQ2 slices
      ident [128, 33]   f32   identity blocks at partitions 0:33, 64:97
    Output:
      out   [NQL, 32]   f32   attention output (n, d) for the queries
    """
    Exp = mybir.ActivationFunctionType.Exp

    nc = bacc.Bacc()
    xs_d = nc.declare_dram_parameter("xs", [C, NT], BF16, isOutput=False)
    xq_d = nc.declare_dram_parameter("xq", [C, NQL], BF16, isOutput=False)
    w_d = nc.declare_dram_parameter("w", [C, 96], BF16, isOutput=False)
    id_d = nc.declare_dram_parameter("ident", [C, 33], F32, isOutput=False)
    out_d = nc.declare_dram_parameter("out", [NQL, HD], F32, isOutput=True)

    with tile.TileContext(nc) as tc:
        with (
            tc.tile_pool(name="consts", bufs=1) as consts,
            tc.tile_pool(name="mains", bufs=1) as mains,
        ):
            w_sb = consts.tile([C, 96], BF16)
            nc.sync.dma_start(out=w_sb[:, :], in_=w_d[:, :])
            id_sb = consts.tile([C, 33], F32)
            nc.sync.dma_start(out=id_sb[:, :], in_=id_d[:, :])

            def chunked_dma(eng, dst, src, total):
                sizes, rem = [], total
                for sz in (512, 512, 1024):
                    if rem >= sz:
                        sizes.append(sz)
                        rem -= sz
                while rem > 0:
                    sz = 2048 if rem >= 2048 else 512
                    sizes.append(sz)
                    rem -= sz
                off = 0
                for sz in sizes:
                    eng.dma_start(out=dst[:, ds(off, sz)],
                                  in_=src[:, ds(off, sz)])
                    off += sz

            xs_sb = mains.tile([C, NT], BF16)
            chunked_dma(nc.gpsimd, xs_sb, xs_d, NT)
            xq_sb = mains.tile([C, NQL], BF16)
            chunked_dma(nc.scalar, xq_sb, xq_d, NQL)

            kk_sb = mains.tile([C, NT], BF16)    # parts 32:48 k1, 64:80 k2
            qq_sb = mains.tile([C, NQL], BF16)   # parts 32:48 q1, 64:80 q2
            vTa_sb = mains.tile([C, MC * 33], BF16)  # per chunk: v^T | ones
            av_sb = mains.tile([C, 2 * NBS], F32)  # parts 0:33 AV1|s1, 64:97 AV2|s2
            out_sb = mains.tile([C, (NQL // 128) * HD], F32)

            nc.vector.memset(vTa_sb[:, :], 1.0)

            with (
                tc.tile_pool(name="sj_ps", bufs=3, space="PSUM") as spool,
                tc.tile_pool(name="av_ps", bufs=1, space="PSUM") as avpool,
                tc.tile_pool(name="e_sb", bufs=18) as epool,
                tc.tile_pool(name="fin_sb", bufs=2) as fsb,
            ):
                def project_q(t):
                    # q chunk t = queries for j-block t
                    ps_q = spool.tile([C, 1024], F32, tag="sj", name="ps_q")
                    nc.tensor.matmul(ps_q[32:48, 0:512], lhsT=w_sb[:, WQ1],
                                     rhs=xq_sb[:, ts(t, 512)],
                                     start=True, stop=True)
                    nc.tensor.matmul(ps_q[64:80, 0:512], lhsT=w_sb[:, WQ2],
                                     rhs=xq_sb[:, ts(t, 512)],
                                     start=True, stop=True)
                    nc.vector.tensor_copy(qq_sb[32:48, ts(t, 512)],
                                          ps_q[32:48, 0:512])
                    nc.vector.tensor_copy(qq_sb[64:80, ts(t, 512)],
                                          ps_q[64:80, 0:512])

                def project_kv(t):
                    # k chunk t = keys for m-chunks 4t..4t+3
                    ps_kv = spool.tile([C, 1024], F32, tag="sj", name="ps_kv")
                    nc.tensor.matmul(ps_kv[32:48, 0:512], lhsT=w_sb[:, WK1],
                                     rhs=xs_sb[:, ts(t, 512)],
                                     start=True, stop=True)
                    nc.tensor.matmul(ps_kv[64:80, 0:512], lhsT=w_sb[:, WK2],
                                     rhs=xs_sb[:, ts(t, 512)],
                                     start=True, stop=True)
                    nc.vector.tensor_copy(kk_sb[32:48, ts(t, 512)],
                                          ps_kv[32:48, 0:512])
                    nc.vector.tensor_copy(kk_sb[64:80, ts(t, 512)],
                                          ps_kv[64:80, 0:512])

                def project_vt(m):
                    ps_vt = spool.tile([C, 1024], F32, tag="sj", name="ps_vt")
                    nc.tensor.matmul(ps_vt[:, 0:HD], lhsT=xs_sb[:, ts(m, 128)],
                                     rhs=w_sb[:, WV], start=True, stop=True)
                    nc.scalar.copy(vTa_sb[:, ds(m * 33, HD)], ps_vt[:, 0:HD])

                def finalize_nb(nb):
                    # transpose av -> [n, 33], reciprocal of the sum
                    # column, combine out = av1/s1 - 0.1*av2/s2 on DVE
                    CQ = NBS // 128  # 8 query chunks of 128
                    psT1 = spool.tile([C, 1024], F32, tag="sj", name="psT1")
                    psT2 = spool.tile([C, 1024], F32, tag="sj", name="psT2")
                    for cq in range(CQ):
                        gq = nb * CQ + cq
                        nc.tensor.transpose(psT1[:, ds(cq * 64, 33)],
                                            av_sb[0:33, ts(gq, 128)],
                                            id_sb[0:33, :])
                        nc.tensor.transpose(psT2[:, ds(cq * 64, 33)],
                                            av_sb[64:97, ts(gq, 128)],
                                            id_sb[64:97, :])
                    r1_sb = fsb.tile([C, CQ], F32, tag="r1")
                    r2_sb = fsb.tile([C, CQ], F32, tag="r2")
                    sum1 = psT1[:, 0:CQ * 64].rearrange(
                        "p (c x) -> p c x", x=64)[:, :, 32:33]
                    sum2 = psT2[:, 0:CQ * 64].rearrange(
                        "p (c x) -> p c x", x=64)[:, :, 32:33]
                    nc.vector.reciprocal(r1_sb[:, :, None], sum1)
                    nc.vector.reciprocal(r2_sb[:, :, None], sum2)
                    nc.vector.tensor_scalar_mul(r2_sb[:, :], r2_sb[:, :],
                                                -LAMBDA)
                    o1_sb = fsb.tile([C, CQ * HD], F32, tag="o1")
                    o2_sb = fsb.tile([C, CQ * HD], F32, tag="o2")
                    av1t = psT1[:, 0:CQ * 64].rearrange(
                        "p (c x) -> p c x", x=64)[:, :, 0:32]
                    av2t = psT2[:, 0:CQ * 64].rearrange(
                        "p (c x) -> p c x", x=64)[:, :, 0:32]
                    o1_v = o1_sb[:, :].rearrange("p (c d) -> p c d", d=HD)
                    o2_v = o2_sb[:, :].rearrange("p (c d) -> p c d", d=HD)
                    nc.vector.tensor_tensor(
                        o1_v, av1t,
                        r1_sb[:, :, None].to_broadcast((C, CQ, HD)),
                        mybir.AluOpType.mult)
                    nc.vector.tensor_tensor(
                        o2_v, av2t,
                        r2_sb[:, :, None].to_broadcast((C, CQ, HD)),
                        mybir.AluOpType.mult)
                    nc.vector.tensor_tensor(
                        out_sb[:, ds(nb * CQ * HD, CQ * HD)],
                        o1_sb[:, :], o2_sb[:, :], mybir.AluOpType.add)
                    out_view = out_d[:, :].rearrange("(c p) d -> p c d", p=C)
                    nc.sync.dma_start(
                        out=out_view[:, nb * CQ:(nb + 1) * CQ, :],
                        in_=out_sb[:, ds(nb * CQ * HD, CQ * HD)]
                            .rearrange("p (c d) -> p c d", d=HD),
                    )

                # minimal chain to the first scores: k chunks 0,1 and
                # the first j-block's queries
                project_kv(0)
                project_q(0)
                project_kv(1)

                pending_av = None      # av batch closure of previous group
                pending_fin = None     # finalize closure of previous n-block

                for j in range(NJ):
                    nb = j // 2
                    for g in range(NG):
                        # ---- boundary work (all PE disturbances live
                        # here): projections one group ahead + finalize
                        if j == 0:
                            if g >= 1:
                                project_kv(2 * g)
                                project_kv(2 * g + 1)
                            # v^T for this group's m-range (consumed by
                            # this group's av batch)
                            for m in range(g * GB, g * GB + GB):
                                project_vt(m)
                        if j == 0 and g == 2:
                            project_q(1)
                        if j == 1 and g == 2:
                            project_q(2)
                        if j == 2 and g == 0 and pending_fin is not None:
                            pending_fin()
                            pending_fin = None
                        if j == 2 and g == 2:
                            project_q(3)

                        # ---- s batch: 8 score pairs + split exps
                        ca = CA_J0 if j == 0 else CA
                        e_tiles = []
                        for m in range(g * GB, g * GB + GB):
                            nsl = ds(j * 512, 512)
                            sj_ps = spool.tile([C, 1024], F32, tag="sj")
                            nc.tensor.matmul(sj_ps[:, 0:512],
                                             lhsT=kk_sb[32:48, ts(m, 128)],
                                             rhs=qq_sb[32:48, nsl],
                                             start=True, stop=True)
                            nc.tensor.matmul(sj_ps[:, 512:1024],
                                             lhsT=kk_sb[64:80, ts(m, 128)],
                                             rhs=qq_sb[64:80, nsl],
                                             start=True, stop=True)
                            e_sb = epool.tile([C, 1024], BF16, tag="e")
                            nc.scalar.activation(e_sb[:, 0:ca],
                                                 sj_ps[:, 0:ca], Exp,
                                                 scale=SCALE)
                            nc.vector.tensor_scalar(
                                e_sb[:, ca:1024].bitcast(I16),
                                sj_ps[:, ca:1024], SCH_A, SCH_B,
                                mybir.AluOpType.mult, mybir.AluOpType.add)
                            e_tiles.append((m, e_sb))
                            last_sj = sj_ps

                        # ---- release gate + previous group's av batch.
                        # The gate (a value-preserving bypass touching the
                        # pending batch's vTa rows, reading this batch's
                        # last sj tile) makes all 16 avs become ready
                        # ATOMICALLY after this s batch finishes: they
                        # run as one uniform burst instead of being
                        # picked one-by-one into the s batch's exp-paced
                        # stalls by the scheduler's ready-heap.
                        if pending_av is not None:
                            fn, m_lo = pending_av
                            nc.vector.tensor_tensor(
                                vTa_sb[0:1, ds(m_lo * 33, GB * 33)],
                                vTa_sb[0:1, ds(m_lo * 33, GB * 33)],
                                last_sj[0:1, 0:1].to_broadcast(
                                    (1, GB * 33)),
                                mybir.AluOpType.bypass)
                            fn()
                            pending_av = None

                        def av_batch(e_tiles=e_tiles, j=j, av_ps=None):
                            for m, e_sb in e_tiles:
                                first, last = (m % MC == 0), (m % MC == MC - 1)
                                nc.tensor.matmul(
                                    av_ps[0:33, ts(j % 2, 512)],
                                    lhsT=vTa_sb[:, ds(m * 33, 33)],
                                    rhs=e_sb[:, 0:512],
                                    start=first, stop=last,
                                    skip_group_check=True)
                                nc.tensor.matmul(
                                    av_ps[64:97, ts(j % 2, 512)],
                                    lhsT=vTa_sb[:, ds(m * 33, 33)],
                                    rhs=e_sb[:, 512:1024],
                                    start=first, stop=last,
                                    skip_group_check=True)
                        if j % 2 == 0 and g == 0:
                            av_tile = avpool.tile([C, NBS], F32, tag="av")
                        pending_av = ((lambda f=av_batch, t=av_tile:
                                       f(av_ps=t)), g * GB)

                    # ---- end of j-block
                    if j % 2 == 1:
                        # flush the block's last av batch, then drain
                        pending_av[0]()
                        pending_av = None
                        nc.vector.tensor_copy(
                            av_sb[0:33, ds(nb * NBS, NBS)], av_tile[0:33, :])
                        nc.scalar.copy(
                            av_sb[64:97, ds(nb * NBS, NBS)], av_tile[64:97, :])
                        if j == NJ - 1:
                            finalize_nb(nb)
                        else:
                            pending_fin = (lambda nb=nb: finalize_nb(nb))

    nc.compile()
    return nc


def make_identity_input():
    ident = np.zeros((C, 33), np.float32)
    ident[0:33, :] = np.eye(33, dtype=np.float32)
    ident[64:97, :] = np.eye(33, dtype=np.float32)
    return ident


def make_in_maps(x, w_qkv):
    """Host-side sharding: subsample, pack per-core inputs."""
    xs = np.ascontiguousarray(x[0][:, ::R, ::R, ::R]).reshape(C, N)
    xs_b = xs.astype(NP_BF16)
    ident = make_identity_input()
    in_maps = []
    for core in range(N_CORES):
        h, half = divmod(core, 2)
        wq = w_qkv[h * 96: h * 96 + 32]       # [32, 128]
        wk = w_qkv[h * 96 + 32: h * 96 + 64]
        wv = w_qkv[h * 96 + 64: h * 96 + 96]
        w = np.empty((C, 96), np.float32)
        w[:, WV] = wv.T
        w[:, WK1] = wk[0:DH].T
        w[:, WK2] = wk[DH:HD].T
        w[:, WQ1] = wq[0:DH].T
        w[:, WQ2] = wq[DH:HD].T
        in_maps.append({
            "xs": xs_b,
            "xq": np.ascontiguousarray(xs_b[:, half * NQ:(half + 1) * NQ]),
            "w": w.astype(NP_BF16),
            "ident": ident,
        })
    return in_maps


_NC_CACHE = {}


def get_nc():
    if "nc" not in _NC_CACHE:
        _NC_CACHE["nc"] = build_nc()
    return _NC_CACHE["nc"]


LAST_RESULTS = None  # BassKernelResults of the most recent kernel() call


def kernel(x, w_qkv, trace=False, **trace_kwargs):
    global LAST_RESULTS
    x = np.asarray(x)
    w_qkv = np.asarray(w_qkv)
    in_maps = make_in_maps(x, w_qkv)
    nc = get_nc()
    res = run_bass_kernel_spmd(nc, in_maps, list(range(N_CORES)),
                               trace=trace, **trace_kwargs)
    LAST_RESULTS = res
    out_hnd = np.empty((HEADS, N, HD), np.float32)
    for core in range(N_CORES):
        h, half = divmod(core, 2)
        out_hnd[h, half * NQ:(half + 1) * NQ, :] = res.results[core]["out"]
    return out_hnd.reshape(1, C, 16, 16, 16)
